# revision 3
# baseline (speedup 1.0000x reference)
"""Differentiable-JPEG forward kernel for 8 Trainium2 NeuronCores.

Strategy (pure data parallel, one image per core):
  RGB->YCbCr + offsets are folded into pass-A matmuls (lhsT = Mfwd[c,c']*BD(D^T),
  plus a K=1 rank-1 matmul for the +0.5 chroma offset). The 8x8 DCT is applied
  with 128x128 block-diagonal DCT matrices; the h<->w layout swap between the
  two DCT directions uses the DVE 32x32 StreamTranspose, which composes with
  the block-diagonal matmuls so only 32-blocked ("Z-layout") transposes are
  ever needed. The 20-sigmoid soft-round collapses to floor(x) + sigmoid(
  50*frac(x) - 25) (tail terms < 2e-11). Only the quantization *correction*
  E = IDCT(delta*qt) flows through the inverse passes (identity path:
  out = clip(X + Minv @ E)), so the post-sigmoid passes can use fp32r
  (11-fraction-bit) matmuls without precision risk; the forward DCT pass B
  stays fp32.
"""
import sys
sys.path.insert(0, '/opt/trn_rl_repo')
import numpy as np
from contextlib import ExitStack

H = W = 512
NCORES = 8
TEMP = 50.0
MAGIC = 12582912.0  # 1.5 * 2^23

MFWD = np.array([[0.299, 0.587, 0.114],
                 [-0.168736, -0.331264, 0.5],
                 [0.5, -0.418688, -0.081312]], dtype=np.float64)
OFFS = np.array([0.0, 0.5, 0.5], dtype=np.float64)
MINV = np.array([[1.0, 0.0, 1.402],
                 [1.0, -0.344136, -0.714136],
                 [1.0, 1.772, 0.0]], dtype=np.float64)


def _dct8():
    n = np.arange(8)
    mat = np.sqrt(2.0 / 8) * np.cos((2 * n[None, :] + 1) * n[:, None] * np.pi / 16.0)
    mat[0, :] = np.sqrt(1.0 / 8)
    return mat


_D8 = _dct8()
_BD = np.kron(np.eye(16), _D8)       # 128x128, block diag of D
_BDT = _BD.T.copy()

_MODULE_CACHE = {}


def _build_module(reps=1):
    import concourse.bass as bass
    import concourse.tile as tile
    from concourse import bacc, mybir

    F32 = mybir.dt.float32
    F32R = mybir.dt.float32r
    ALU = mybir.AluOpType
    AF = mybir.ActivationFunctionType

    nc = bacc.Bacc("TRN2", target_bir_lowering=False, debug=False,
                   num_devices=NCORES)

    img_d = nc.dram_tensor("img", [3, H, W], F32, kind="ExternalInput").ap()
    wa_d = nc.dram_tensor("wa", [128, 9 * 128], F32, kind="ExternalInput").ap()
    wb_d = nc.dram_tensor("wb", [128, 128], F32, kind="ExternalInput").ap()
    wc_d = nc.dram_tensor("wc", [128, 128], F32, kind="ExternalInput").ap()
    wd_d = nc.dram_tensor("wd", [128, 10 * 128], F32, kind="ExternalInput").ap()
    offa_d = nc.dram_tensor("offa", [1, 128], F32, kind="ExternalInput").ap()
    ones_d = nc.dram_tensor("ones", [1, 512], F32, kind="ExternalInput").ap()
    qtinv_d = nc.dram_tensor("qtinv", [128, 1536], F32, kind="ExternalInput").ap()
    qt_d = nc.dram_tensor("qt", [128, 1536], F32, kind="ExternalInput").ap()
    out_d = nc.dram_tensor("out", [3, H, W], F32, kind="ExternalOutput").ap()

    with tile.TileContext(nc) as tc, ExitStack() as ctx:
        const = ctx.enter_context(tc.tile_pool(name="const", bufs=1))
        imgp = ctx.enter_context(tc.tile_pool(name="imgp", bufs=6))
        sb_a = ctx.enter_context(tc.tile_pool(name="sb_a", bufs=2))
        sb_z = ctx.enter_context(tc.tile_pool(name="sb_z", bufs=2))
        sb_q = ctx.enter_context(tc.tile_pool(name="sb_q", bufs=2))
        sb_g = ctx.enter_context(tc.tile_pool(name="sb_g", bufs=2))
        sb_u = ctx.enter_context(tc.tile_pool(name="sb_u", bufs=3))
        sb_o = ctx.enter_context(tc.tile_pool(name="sb_o", bufs=3))
        pa = ctx.enter_context(tc.tile_pool(name="pa", bufs=2, space="PSUM"))
        pb = ctx.enter_context(tc.tile_pool(name="pb", bufs=2, space="PSUM"))
        pc = ctx.enter_context(tc.tile_pool(name="pc", bufs=2, space="PSUM"))
        pd = ctx.enter_context(tc.tile_pool(name="pd", bufs=2, space="PSUM"))

        wa = const.tile([128, 9 * 128], F32R, tag="wa")
        nc.sync.dma_start(wa[:], wa_d.bitcast(F32R))
        wb = const.tile([128, 128], F32, tag="wb")
        nc.sync.dma_start(wb[:], wb_d)
        wc = const.tile([128, 128], F32R, tag="wc")
        nc.sync.dma_start(wc[:], wc_d.bitcast(F32R))
        wd = const.tile([128, 10 * 128], F32R, tag="wd")
        nc.sync.dma_start(wd[:], wd_d.bitcast(F32R))
        offa = const.tile([1, 128], F32R, tag="offa")
        nc.sync.dma_start(offa[:], offa_d.bitcast(F32R))
        qtinv = const.tile([128, 1536], F32, tag="qtinv")
        nc.sync.dma_start(qtinv[:], qtinv_d)
        qt = const.tile([128, 1536], F32, tag="qt")
        nc.sync.dma_start(qt[:], qt_d)
        ones = const.tile([1, 512], F32R, tag="ones")
        nc.sync.dma_start(ones[:], ones_d.bitcast(F32R))
        bm25 = const.tile([128, 1], F32, tag="bm25")
        nc.vector.memset(bm25[:], -25.0)

        for rep in range(reps):
            for t in range(4):
                hs = slice(t * 128, (t + 1) * 128)
                imgs = []
                for c in range(3):
                    im = imgp.tile([128, 512], F32R, tag="img")
                    nc.sync.dma_start(im[:], img_d[c, hs, :].bitcast(F32R))
                    imgs.append(im)

                # pass A (fp32r, color+offset folded) -> evac -> ST1 -> pass B
                z1s = []
                for c in range(3):
                    pA = pa.tile([128, 512], F32, tag="pa")
                    for c2 in range(3):
                        nc.tensor.matmul(pA[:], wa[:, (3 * c + c2) * 128:
                                                    (3 * c + c2 + 1) * 128],
                                         imgs[c2][:],
                                         start=(c2 == 0),
                                         stop=(c2 == 2 and c == 0))
                    if c != 0:
                        nc.tensor.matmul(pA[:], offa[:], ones[:],
                                         start=False, stop=True)
                    a1 = sb_a.tile([128, 512], F32, tag="a1")
                    nc.scalar.copy(a1[:], pA[:])
                    z1 = sb_z.tile([128, 512], F32, tag="z1")
                    nc.vector.transpose(z1[:], a1[:])
                    z1s.append(z1)

                xt = sb_q.tile([128, 1536], F32, tag="xt")
                pBs = []
                for c in range(3):
                    pB = pb.tile([128, 512], F32, tag="pb")
                    nc.tensor.matmul(pB[:], wb[:], z1s[c][:],
                                     start=True, stop=True)
                    pBs.append(pB)
                    csl = slice(c * 512, (c + 1) * 512)
                    nc.vector.tensor_tensor(xt[:, csl], pB[:], qtinv[:, csl],
                                            op=ALU.mult)

                # soft-round correction: g = (sigmoid(50*frac-25) - frac) * qt
                tt = sb_q.tile([128, 1536], F32, tag="tt")
                nc.gpsimd.tensor_scalar(tt[:], xt[:], -0.5, MAGIC,
                                        op0=ALU.add, op1=ALU.add)
                fl = sb_q.tile([128, 1536], F32, tag="fl")
                nc.gpsimd.tensor_scalar(fl[:], tt[:], MAGIC, None,
                                        op0=ALU.subtract)
                mm = sb_q.tile([128, 1536], F32, tag="mm")
                nc.vector.tensor_tensor(mm[:], xt[:], fl[:], op=ALU.subtract)
                ss = sb_q.tile([128, 1536], F32, tag="ss")
                nc.scalar.activation(ss[:], mm[:], AF.Sigmoid,
                                     bias=bm25[:], scale=TEMP)
                dd = sb_q.tile([128, 1536], F32, tag="dd")
                nc.vector.tensor_tensor(dd[:], ss[:], mm[:], op=ALU.subtract)
                gg = sb_g.tile([128, 1536], F32R, tag="gg")
                nc.vector.tensor_tensor(gg[:], dd[:], qt[:], op=ALU.mult)

                # pass C (fp32r) -> ST2 -> f32r cast -> pass D (fp32r) -> clip
                urs = []
                for c in range(3):
                    pC = pc.tile([128, 512], F32, tag="pc")
                    nc.tensor.matmul(pC[:], wc[:],
                                     gg[:, c * 512:(c + 1) * 512],
                                     start=True, stop=True)
                    u = sb_u.tile([128, 512], F32, tag="u")
                    nc.vector.transpose(u[:], pC[:])
                    ur = sb_u.tile([128, 512], F32R, tag="ur")
                    nc.scalar.copy(ur[:], u[:])
                    urs.append(ur)

                for c in range(3):
                    pD = pd.tile([128, 512], F32, tag="pd")
                    for c2 in range(3):
                        nc.tensor.matmul(pD[:], wd[:, (3 * c + c2) * 128:
                                                    (3 * c + c2 + 1) * 128],
                                         urs[c2][:],
                                         start=(c2 == 0), stop=False)
                    nc.tensor.matmul(pD[:], wd[:, 9 * 128:10 * 128],
                                     imgs[c][:], start=False, stop=True)
                    o = sb_o.tile([128, 512], F32, tag="o")
                    nc.vector.tensor_scalar(o[:], pD[:], 0.0, 1.0,
                                            op0=ALU.max, op1=ALU.min)
                    nc.sync.dma_start(out_d[c, hs, :], o[:])

    nc.compile()
    return nc


def _host_arrays(q_y, q_c):
    qy = np.clip(q_y.astype(np.float64), 2.0, 15.0)
    qc = np.clip(q_c.astype(np.float64), 2.0, 15.0)
    qts = [qy, qc, qc]

    wa = np.zeros((128, 9 * 128), np.float32)
    wd = np.zeros((128, 10 * 128), np.float32)
    for c in range(3):
        for c2 in range(3):
            wa[:, (3 * c + c2) * 128:(3 * c + c2 + 1) * 128] = \
                (MFWD[c, c2] * _BDT).astype(np.float32)
            wd[:, (3 * c + c2) * 128:(3 * c + c2 + 1) * 128] = \
                (MINV[c, c2] * _BD).astype(np.float32)
    wd[:, 9 * 128:] = np.eye(128, dtype=np.float32)
    wb = _BD.astype(np.float32)
    wc = _BDT.astype(np.float32)
    offa = (0.5 * np.sqrt(8.0) *
            (np.arange(128) % 8 == 0)).astype(np.float32)[None, :]

    p_idx = np.arange(128) % 8
    f_idx = np.arange(512) % 8
    qtinv = np.zeros((128, 1536), np.float32)
    qt = np.zeros((128, 1536), np.float32)
    for c in range(3):
        rep = qts[c][np.ix_(f_idx, p_idx)].T
        qt[:, c * 512:(c + 1) * 512] = rep.astype(np.float32)
        qtinv[:, c * 512:(c + 1) * 512] = (1.0 / rep).astype(np.float32)
    return dict(wa=wa, wb=wb, wc=wc, wd=wd, offa=offa, qtinv=qtinv, qt=qt,
                ones=np.ones((1, 512), np.float32))


def kernel(images, q_y, q_c, _reps=1, _time_only=False):
    from concourse.bass_utils import run_bass_kernel_spmd

    images = np.ascontiguousarray(np.asarray(images, dtype=np.float32))
    shared = _host_arrays(np.asarray(q_y), np.asarray(q_c))

    if _reps not in _MODULE_CACHE:
        _MODULE_CACHE[_reps] = _build_module(reps=_reps)
    nc = _MODULE_CACHE[_reps]

    in_maps = [dict(img=np.ascontiguousarray(images[i]), **shared)
               for i in range(NCORES)]
    res = run_bass_kernel_spmd(nc, in_maps, list(range(NCORES)))
    if _time_only:
        return None
    out = np.stack([res.results[i]["out"] for i in range(NCORES)], axis=0)
    return out.astype(np.float32)
